# revision 23
# baseline (speedup 1.0000x reference)
"""Trainium2 Bass kernel for nn_EncoderLayer_85100482003492 (sparse graph attention).

Sharding: 8 cores = (batch b in 0..3) x (query-half sh in 0..1).
Each core handles batch b, queries [sh*2048, (sh+1)*2048), ALL 8 heads:
  - computes K,V for all 4096 tokens of its batch (dup across the pair),
    stores them interleaved as bf16 rows kv[t] = [K(512)|V(512)] in DRAM,
  - indirect-DMA gathers the 32 neighbor KV rows per query (8 x 512-row
    calls per 128-query tile; with 32KB SWDGE descriptor scratch two calls
    fit the ring so desc-gen overlaps the previous call's drain),
  - fused per-query-tile pipeline: dot-products + score tree on DVE (bf16
    2x, in-place in the gather buffer), per-half exp-broadcast expansion on
    ACT (x0.125 scale, starts as soon as that half's scores are ready),
    weighted V sum + tree on DVE (in-place), ctx normalized once by 1/den,
    then WO matmul + LN1 + FFN + LN2 in the same loop so PE/ACT work
    overlaps the next tile's gather DMA.
Host-detected fast paths: when all biases are zero (zb) and LN gamma/beta
are identity (zg) - always true for this problem's setup_inputs - bias
adds become ACT copies and the LN affine folds into the ACT normalize op.
The general path is kept and selected automatically otherwise.
No collectives: each core's output rows are disjoint; host concatenates.
"""
import os
import sys

sys.path.insert(0, "/opt/trn_rl_repo")

import numpy as np

B, S, D, H, DFF, DEG = 4, 4096, 512, 8, 2048, 32
DH = D // H
SH = S // 2          # queries per core
P = 128
NQT = SH // P        # 16 query tiles per core
NTT = S // P         # 32 token tiles
HJ = DEG // 2        # 16 neighbors per gather half
EPS = 1e-6
N_CORES = 8

_compiled = None
LAST_RESULT = None


def _build(zb=False, zg=False):
    import concourse.bacc as bacc
    import concourse.mybir as mybir
    import concourse.tile as tile
    from concourse.library_config import mlp
    from concourse.masks import make_identity

    f32 = mybir.dt.float32
    bf16 = mybir.dt.bfloat16
    ALU = mybir.AluOpType
    ACTF = mybir.ActivationFunctionType

    nc = bacc.Bacc("TRN2", target_bir_lowering=False, debug=False,
                   dynamic_dma_scratch_size=32768)

    x = nc.dram_tensor("x", [S, D], f32, kind="ExternalInput")
    offs = nc.dram_tensor("offs", [P, NQT * 2 * P], mybir.dt.int16, kind="ExternalInput")
    xbf = nc.dram_tensor("xbf", [S, D], bf16, kind="ExternalInput")
    wq = nc.dram_tensor("wq", [D, D], bf16, kind="ExternalInput")
    wk = nc.dram_tensor("wk", [D, D], bf16, kind="ExternalInput")
    wv = nc.dram_tensor("wv", [D, D], bf16, kind="ExternalInput")
    wo = nc.dram_tensor("wo", [D, D], bf16, kind="ExternalInput")
    w1 = nc.dram_tensor("w1", [D, DFF], bf16, kind="ExternalInput")
    w2 = nc.dram_tensor("w2", [DFF, D], bf16, kind="ExternalInput")
    # host-prebroadcast bias/ln tensors
    bq_b = nc.dram_tensor("bq_b", [P, D], f32, kind="ExternalInput")
    bk_b = nc.dram_tensor("bk_b", [P, D], f32, kind="ExternalInput")
    bv_b = nc.dram_tensor("bv_b", [P, D], f32, kind="ExternalInput")
    bo_b = nc.dram_tensor("bo_b", [P, D], f32, kind="ExternalInput")
    b2_b = nc.dram_tensor("b2_b", [P, D], f32, kind="ExternalInput")
    g1_b = nc.dram_tensor("g1_b", [P, D], f32, kind="ExternalInput")
    bt1_b = nc.dram_tensor("bt1_b", [P, D], f32, kind="ExternalInput")
    g2_b = nc.dram_tensor("g2_b", [P, D], f32, kind="ExternalInput")
    bt2_b = nc.dram_tensor("bt2_b", [P, D], f32, kind="ExternalInput")
    b1t = nc.dram_tensor("b1t", [P, DFF // P], f32, kind="ExternalInput")

    out = nc.dram_tensor("out", [SH, D], f32, kind="ExternalOutput")

    nc.gpsimd.load_library(mlp)
    with tile.TileContext(nc) as tc:
        with (
            tc.tile_pool(name="dram", bufs=1, space="DRAM") as dram_pool,
            tc.tile_pool(name="persist", bufs=1) as persist,
        ):
            kv_dram = dram_pool.tile([S, 2 * D], bf16)
            q_dram = dram_pool.tile([SH, D], bf16)

            ident = persist.tile([P, P], bf16)
            make_identity(nc, ident[:])
            eps_t = persist.tile([P, 1], f32)
            nc.vector.memset(eps_t[:], EPS)
            # persistent weights for the fused attention+FFN loop
            wo_s = persist.tile([P, 4, D], bf16)
            nc.sync.dma_start(
                out=wo_s[:], in_=wo.ap()[:].rearrange("(a p) d -> p a d", p=P)
            )
            w1_s = persist.tile([P, 4, DFF], bf16)
            nc.sync.dma_start(
                out=w1_s[:], in_=w1.ap()[:].rearrange("(a p) f -> p a f", p=P)
            )
            w2_s = persist.tile([P, 16, D], bf16)
            nc.sync.dma_start(
                out=w2_s[:], in_=w2.ap()[:].rearrange("(a p) d -> p a d", p=P)
            )
            bos = b2s = g1s = bt1s = g2s = bt2s = None
            if not zb:
                bos = persist.tile([P, D], f32)
                b2s = persist.tile([P, D], f32)
                nc.sync.dma_start(out=bos[:], in_=bo_b.ap()[:])
                nc.sync.dma_start(out=b2s[:], in_=b2_b.ap()[:])
            if not zg:
                g1s = persist.tile([P, D], f32)
                bt1s = persist.tile([P, D], f32)
                g2s = persist.tile([P, D], f32)
                bt2s = persist.tile([P, D], f32)
                nc.sync.dma_start(out=g1s[:], in_=g1_b.ap()[:])
                nc.sync.dma_start(out=bt1s[:], in_=bt1_b.ap()[:])
                nc.sync.dma_start(out=g2s[:], in_=g2_b.ap()[:])
                nc.sync.dma_start(out=bt2s[:], in_=bt2_b.ap()[:])
            b1t_s = persist.tile([P, DFF // P], f32)
            nc.sync.dma_start(out=b1t_s[:], in_=b1t.ap()[:])

            # ---------------- Phase 1: xT, QKV projections, KV store -------
            with (
                tc.tile_pool(name="p1sb", bufs=3) as p1sb,
                tc.tile_pool(name="p1w", bufs=1) as p1w,
                tc.tile_pool(name="p1psmm", bufs=2, space="PSUM") as p1psmm,
            ):
                wq_s = p1w.tile([P, 4, D], bf16)
                wk_s = p1w.tile([P, 4, D], bf16)
                wv_s = p1w.tile([P, 4, D], bf16)
                nc.sync.dma_start(
                    out=wq_s[:], in_=wq.ap()[:].rearrange("(a p) d -> p a d", p=P)
                )
                nc.sync.dma_start(
                    out=wk_s[:], in_=wk.ap()[:].rearrange("(a p) d -> p a d", p=P)
                )
                nc.sync.dma_start(
                    out=wv_s[:], in_=wv.ap()[:].rearrange("(a p) d -> p a d", p=P)
                )
                bqs = p1w.tile([P, D], f32)
                bks = p1w.tile([P, D], f32)
                bvs = p1w.tile([P, D], f32)
                nc.sync.dma_start(out=bqs[:], in_=bq_b.ap()[:])
                nc.sync.dma_start(out=bks[:], in_=bk_b.ap()[:])
                nc.sync.dma_start(out=bvs[:], in_=bv_b.ap()[:])

                xT = p1w.tile([P, 4, S], bf16)  # [d%128, d//128, t]
                for dt in range(4):
                    nc.sync.dma_start(
                        out=xT[:, dt, :],
                        in_=xbf.ap()[:, dt * P : (dt + 1) * P],
                        transpose=True,
                    )

                for tt in range(NTT):
                    kv_stage = p1sb.tile([P, 2 * D], bf16, tag="kvst")
                    kps = p1psmm.tile([P, D], f32, tag="kps")
                    for dt in range(4):
                        nc.tensor.matmul(
                            out=kps[:],
                            lhsT=xT[:, dt, tt * P : (tt + 1) * P],
                            rhs=wk_s[:, dt, :],
                            start=(dt == 0),
                            stop=(dt == 3),
                        )
                    if zb:
                        nc.scalar.copy(out=kv_stage[:, 0:D], in_=kps[:])
                    else:
                        nc.vector.tensor_tensor(
                            out=kv_stage[:, 0:D], in0=kps[:], in1=bks[:], op=ALU.add
                        )
                    vps = p1psmm.tile([P, D], f32, tag="kps")
                    for dt in range(4):
                        nc.tensor.matmul(
                            out=vps[:],
                            lhsT=xT[:, dt, tt * P : (tt + 1) * P],
                            rhs=wv_s[:, dt, :],
                            start=(dt == 0),
                            stop=(dt == 3),
                        )
                    if zb:
                        nc.scalar.copy(out=kv_stage[:, D : 2 * D], in_=vps[:])
                    else:
                        nc.vector.tensor_tensor(
                            out=kv_stage[:, D : 2 * D], in0=vps[:], in1=bvs[:],
                            op=ALU.add,
                        )
                    nc.sync.dma_start(
                        out=kv_dram[tt * P : (tt + 1) * P, :], in_=kv_stage[:]
                    )

                # Q for own half (token tiles [16, 32) after host rotation)
                for qt in range(NQT):
                    tcol = _Q0_TILE + qt
                    qps = p1psmm.tile([P, D], f32, tag="kps")
                    for dt in range(4):
                        nc.tensor.matmul(
                            out=qps[:],
                            lhsT=xT[:, dt, tcol * P : (tcol + 1) * P],
                            rhs=wq_s[:, dt, :],
                            start=(dt == 0),
                            stop=(dt == 3),
                        )
                    q_stage = p1sb.tile([P, D], bf16, tag="qst")
                    if zb:
                        nc.scalar.copy(out=q_stage[:], in_=qps[:])
                    else:
                        nc.vector.tensor_tensor(
                            out=q_stage[:], in0=qps[:], in1=bqs[:], op=ALU.add
                        )
                    nc.sync.dma_start(
                        out=q_dram[qt * P : (qt + 1) * P, :], in_=q_stage[:]
                    )

            # ---------------- Fused attention + FFN loop --------------------
            with (
                tc.tile_pool(name="akv", bufs=3) as akv,
                tc.tile_pool(name="ap64", bufs=2) as ap64,
                tc.tile_pool(name="asmall", bufs=2) as asmall,
                tc.tile_pool(name="abig", bufs=1) as abig,
                tc.tile_pool(name="aps", bufs=2, space="PSUM") as aps,
                tc.tile_pool(name="apsmm", bufs=1, space="PSUM") as apsmm,
                tc.tile_pool(name="bpsh", bufs=3, space="PSUM") as bpsh,
                tc.tile_pool(name="bpsmm", bufs=2, space="PSUM") as bpsmm,
            ):
                for qt in range(NQT):
                    # -------- async KV gathers for this tile (2KB rows) -----
                    offs_t = asmall.tile([P, 4 * 64], mybir.dt.int16, tag="offs")
                    nc.sync.dma_start(
                        out=offs_t[:],
                        in_=offs.ap()[:, qt * 256 : (qt + 1) * 256],
                    )
                    kvgs = []
                    for hf in range(2):
                        kvg = akv.tile([P, HJ, 2 * D], bf16, tag="kvg")
                        for s in range(4):
                            cc = hf * 4 + s
                            nc.gpsimd.dma_gather(
                                kvg[:, s * 4 : (s + 1) * 4, :],
                                kv_dram[:],
                                offs_t[:, cc * 32 : (cc + 1) * 32],
                                P * 4,
                                P * 4,
                                2 * D,
                            )
                        kvgs.append(kvg)

                    q_t = asmall.tile([P, D], bf16, tag="qt")
                    nc.sync.dma_start(
                        out=q_t[:], in_=q_dram[qt * P : (qt + 1) * P, :]
                    )
                    x_t = asmall.tile([P, D], f32, tag="xres")
                    nc.sync.dma_start(
                        out=x_t[:],
                        in_=x.ap()[_Q0_TILE * P + qt * P : _Q0_TILE * P + (qt + 1) * P, :],
                    )
                    if not zb:
                        xpbo = abig.tile([P, D], f32, tag="xpbo")
                        nc.vector.tensor_tensor(
                            out=xpbo[:], in0=x_t[:], in1=bos[:], op=ALU.add
                        )

                    scores = asmall.tile([P, 2, P], f32, tag="scores")
                    for hf in range(2):
                        kvg = kvgs[hf]
                        # prod = Kg * q, in-place into the gathered K half
                        nc.vector.tensor_tensor(
                            out=kvg[:, :, 0:D],
                            in0=kvg[:, :, 0:D],
                            in1=q_t[:]
                            .rearrange("p (o d) -> p o d", o=1)
                            .to_broadcast([P, HJ, D]),
                            op=ALU.mult,
                        )
                        # in-place tree-reduce over dh=64 -> [P, j, h]
                        cur = kvg[:, :, 0:D].rearrange("p j (g d) -> p j g d", d=DH)
                        w = DH
                        while w > 2:
                            half = w // 2
                            nc.vector.tensor_tensor(
                                out=cur[:, :, :, 0:half],
                                in0=cur[:, :, :, 0:half],
                                in1=cur[:, :, :, half:w],
                                op=ALU.add,
                            )
                            w = half
                        nc.vector.tensor_tensor(
                            out=scores[:, hf, :].rearrange(
                                "p (j g o) -> p j g o", g=H, o=1
                            ),
                            in0=cur[:, :, :, 0:1],
                            in1=cur[:, :, :, 1:2],
                            op=ALU.add,
                        )

                    # per-half: e64 = exp-broadcast (starts as soon as this
                    # half's scores are done), weighted V (in-place), tree
                    den = asmall.tile([P, H], f32, tag="den")
                    den_h = asmall.tile([P, H], f32, tag="denh")
                    ctx_halves = []
                    for hf in range(2):
                        kvg = kvgs[hf]
                        e64 = ap64.tile([P, HJ * H, DH], bf16, tag="p64")
                        # split expansion + weighted-V into two j-sub-blocks:
                        # DVE multiplies sub-block a while ACT expands b
                        for sb in range(2):
                            nc.scalar.activation(
                                out=e64[:, sb * 64 : (sb + 1) * 64, :],
                                in_=scores[:, hf, sb * 64 : (sb + 1) * 64]
                                .rearrange("p (a o) -> p a o", o=1)
                                .to_broadcast([P, 64, DH]),
                                func=ACTF.Exp,
                                scale=0.125,
                            )
                            nc.vector.tensor_tensor(
                                out=kvg[:, sb * 8 : (sb + 1) * 8, D : 2 * D],
                                in0=kvg[:, sb * 8 : (sb + 1) * 8, D : 2 * D],
                                in1=e64[:, sb * 64 : (sb + 1) * 64, :]
                                .rearrange("p (j g) d -> p j (g d)", g=H),
                                op=ALU.mult,
                            )
                        nc.vector.tensor_reduce(
                            out=(den if hf == 0 else den_h)[:],
                            in_=e64[:]
                            .rearrange("p (j g) d -> p j g d", g=H)[:, :, :, 0:1]
                            .rearrange("p j g o -> p g (j o)"),
                            axis=mybir.AxisListType.X,
                            op=ALU.add,
                        )
                        w = HJ
                        while w > 2:
                            half = w // 2
                            nc.vector.tensor_tensor(
                                out=kvg[:, 0:half, D : 2 * D],
                                in0=kvg[:, 0:half, D : 2 * D],
                                in1=kvg[:, half:w, D : 2 * D],
                                op=ALU.add,
                            )
                            w = half
                        ctx_halves.append(kvg)
                    nc.vector.tensor_tensor(
                        out=den[:], in0=den[:], in1=den_h[:], op=ALU.add
                    )
                    rden = asmall.tile([P, H], f32, tag="rden")
                    nc.vector.reciprocal(out=rden[:], in_=den[:])

                    ctx_n = abig.tile([P, D], bf16, tag="ctxn")
                    nc.vector.tensor_tensor(
                        out=ctx_n[:],
                        in0=ctx_halves[0][:, 0, D : 2 * D],
                        in1=ctx_halves[0][:, 1, D : 2 * D],
                        op=ALU.add,
                    )
                    nc.vector.tensor_tensor(
                        out=ctx_n[:],
                        in0=ctx_n[:],
                        in1=ctx_halves[1][:, 0, D : 2 * D],
                        op=ALU.add,
                    )
                    nc.vector.tensor_tensor(
                        out=ctx_n[:],
                        in0=ctx_n[:],
                        in1=ctx_halves[1][:, 1, D : 2 * D],
                        op=ALU.add,
                    )
                    nc.vector.tensor_tensor(
                        out=ctx_n[:].rearrange("p (g d) -> p g d", d=DH),
                        in0=ctx_n[:].rearrange("p (g d) -> p g d", d=DH),
                        in1=rden[:]
                        .rearrange("p (g o) -> p g o", o=1)
                        .to_broadcast([P, H, DH]),
                        op=ALU.mult,
                    )

                    # transpose ctx, WO matmul, residual, LN1
                    ctxT = abig.tile([P, 4, P], bf16, tag="ctxT")
                    for dt in range(4):
                        tp = aps.tile([P, P], bf16, tag="tp")
                        nc.tensor.transpose(
                            out=tp[:],
                            in_=ctx_n[:, dt * P : (dt + 1) * P],
                            identity=ident[:],
                        )
                        nc.scalar.copy(out=ctxT[:, dt, :], in_=tp[:])
                    attn = apsmm.tile([P, D], f32, tag="attn")
                    for dt in range(4):
                        nc.tensor.matmul(
                            out=attn[:],
                            lhsT=ctxT[:, dt, :],
                            rhs=wo_s[:, dt, :],
                            start=(dt == 0),
                            stop=(dt == 3),
                        )
                    x1pre = abig.tile([P, D], f32, tag="x1pre")
                    nc.vector.tensor_tensor(
                        out=x1pre[:], in0=attn[:],
                        in1=x_t[:] if zb else xpbo[:], op=ALU.add
                    )
                    x1 = abig.tile([P, D], f32, tag="x1")
                    _layernorm(nc, tc, abig, x1[:], x1pre[:], None if zg else g1s[:], None if zg else bt1s[:], eps_t, ALU, ACTF, f32, identity_gb=zg)

                    # FFN (transposed h layout) + residual + LN2
                    x1b = abig.tile([P, D], bf16, tag="x1b")
                    nc.scalar.copy(out=x1b[:], in_=x1[:])
                    x1T = abig.tile([P, 4, P], bf16, tag="x1T")
                    for dt in range(4):
                        tp = aps.tile([P, P], bf16, tag="tp")
                        nc.tensor.transpose(
                            out=tp[:],
                            in_=x1b[:, dt * P : (dt + 1) * P],
                            identity=ident[:],
                        )
                        nc.scalar.copy(out=x1T[:, dt, :], in_=tp[:])
                    hT = abig.tile([P, 16, P], bf16, tag="hT")
                    for ft in range(16):
                        hps = bpsh.tile([P, P], f32, tag="hps")
                        for dt in range(4):
                            nc.tensor.matmul(
                                out=hps[:],
                                lhsT=w1_s[:, dt, ft * P : (ft + 1) * P],
                                rhs=x1T[:, dt, :],
                                start=(dt == 0),
                                stop=(dt == 3),
                            )
                        nc.scalar.activation(
                            out=hT[:, ft, :],
                            in_=hps[:],
                            func=ACTF.Relu,
                            bias=b1t_s[:, ft : ft + 1],
                        )
                    y2 = bpsmm.tile([P, D], f32, tag="y2")
                    for ft in range(16):
                        nc.tensor.matmul(
                            out=y2[:],
                            lhsT=hT[:, ft, :],
                            rhs=w2_s[:, ft, :],
                            start=(ft == 0),
                            stop=(ft == 15),
                        )
                    x2pre = abig.tile([P, D], f32, tag="x2pre")
                    if zb:
                        nc.vector.tensor_tensor(
                            out=x2pre[:], in0=y2[:], in1=x1[:], op=ALU.add
                        )
                    else:
                        nc.vector.tensor_tensor(
                            out=x2pre[:], in0=y2[:], in1=b2s[:], op=ALU.add
                        )
                        nc.vector.tensor_tensor(
                            out=x2pre[:], in0=x2pre[:], in1=x1[:], op=ALU.add
                        )
                    o_t = abig.tile([P, D], f32, tag="ot")
                    _layernorm(nc, tc, abig, o_t[:], x2pre[:], None if zg else g2s[:], None if zg else bt2s[:], eps_t, ALU, ACTF, f32, identity_gb=zg)
                    nc.sync.dma_start(
                        out=out.ap()[qt * P : (qt + 1) * P, :], in_=o_t[:]
                    )

    nc.compile()
    return nc


def _layernorm(nc, tc, pool, out_ap, in_ap, g_b, bt_b, eps_t, ALU, ACTF, f32,
               identity_gb=False):
    """out = (in - mean)/sqrt(var+EPS) * g + b, per partition over D columns."""
    import concourse.mybir as mybir

    s1 = pool.tile([P, 1], f32, tag="ln_s1")
    nc.vector.tensor_reduce(
        out=s1[:], in_=in_ap, axis=mybir.AxisListType.X, op=ALU.add
    )
    sqd = pool.tile([P, D], f32, tag="ln_scratch")
    s2 = pool.tile([P, 1], f32, tag="ln_s2")
    nc.scalar.activation(
        out=sqd[:], in_=in_ap, func=ACTF.Square, accum_out=s2[:]
    )
    nmean = pool.tile([P, 1], f32, tag="ln_nmean")
    nc.scalar.mul(out=nmean[:], in_=s1[:], mul=-1.0 / D)
    ex2 = pool.tile([P, 1], f32, tag="ln_ex2")
    nc.scalar.mul(out=ex2[:], in_=s2[:], mul=1.0 / D)
    m2 = pool.tile([P, 1], f32, tag="ln_m2")
    nc.vector.tensor_tensor(out=m2[:], in0=nmean[:], in1=nmean[:], op=ALU.mult)
    var = pool.tile([P, 1], f32, tag="ln_var")
    nc.vector.tensor_tensor(out=var[:], in0=ex2[:], in1=m2[:], op=ALU.subtract)
    std = pool.tile([P, 1], f32, tag="ln_std")
    nc.scalar.activation(out=std[:], in_=var[:], func=ACTF.Sqrt, bias=eps_t[:, 0:1])
    rstd = pool.tile([P, 1], f32, tag="ln_rstd")
    nc.vector.reciprocal(out=rstd[:], in_=std[:])
    nmr = pool.tile([P, 1], f32, tag="ln_nmr")
    nc.vector.tensor_tensor(out=nmr[:], in0=nmean[:], in1=rstd[:], op=ALU.mult)
    if identity_gb:
        nc.scalar.activation(
            out=out_ap, in_=in_ap, func=ACTF.Identity, bias=nmr[:, 0:1],
            scale=rstd[:, 0:1],
        )
    else:
        xn = pool.tile([P, D], f32, tag="ln_scratch")
        nc.scalar.activation(
            out=xn[:], in_=in_ap, func=ACTF.Identity, bias=nmr[:, 0:1],
            scale=rstd[:, 0:1],
        )
        nc.vector.tensor_tensor(out=xn[:], in0=xn[:], in1=g_b, op=ALU.mult)
        nc.vector.tensor_tensor(out=out_ap, in0=xn[:], in1=bt_b, op=ALU.add)


# Q-tile offset within the 32 token tiles. Both half-cores share the same
# compiled program; the host passes x ROTATED for sh=0 cores so that the
# query half always sits at token tiles [16, 32). See _prep().
_Q0_TILE = 16


def _prep(inputs):
    x = np.ascontiguousarray(np.asarray(inputs["x"], dtype=np.float32))
    edges = np.asarray(inputs["edges"])
    kidx = np.ascontiguousarray(edges[:, 1].reshape(S, DEG)).astype(np.int32)

    def bb(name):
        return np.ascontiguousarray(
            np.broadcast_to(np.asarray(inputs[name], np.float32), (P, D))
        )

    import ml_dtypes

    def cbf(name):
        return np.ascontiguousarray(
            np.asarray(inputs[name], np.float32).astype(ml_dtypes.bfloat16)
        )

    shared = {
        "wq": cbf("wq"),
        "wk": cbf("wk"),
        "wv": cbf("wv"),
        "wo": cbf("wo"),
        "w1": cbf("w1"),
        "w2": cbf("w2"),
        "bq_b": bb("bq"),
        "bk_b": bb("bk"),
        "bv_b": bb("bv"),
        "bo_b": bb("bo"),
        "b2_b": bb("b2"),
        "g1_b": bb("ln1_g"),
        "bt1_b": bb("ln1_b"),
        "g2_b": bb("ln2_g"),
        "bt2_b": bb("ln2_b"),
        "b1t": np.ascontiguousarray(
            np.asarray(inputs["b1"], np.float32).reshape(DFF // P, P).T
        ),
    }

    # dma_gather wrapped idx layout for a 512-row gather block (qt, hf, s):
    # gathered row i = edge (q = i%128, j = hf*16 + s*4 + i//128); idx value
    # for row i sits at [partition i%16, column i//16], replicated x8 cores.
    # 512-row calls (1024 descriptors) let two calls coexist in the 2048-slot
    # SWDGE ring so desc-gen overlaps the previous call's drain.
    ppidx = (np.arange(32)[None, :] * 16) + (np.arange(P)[:, None] % 16)

    in_maps = []
    for c in range(N_CORES):
        b, sh = c // 2, c % 2
        q0 = sh * SH
        # rotate tokens so this core's queries sit at token tiles [16, 32)
        # (kv gather indices are rotated to match)
        if sh == 0:
            xb = np.concatenate([x[b, SH:], x[b, :SH]], axis=0)
            rot = lambda t: (t + SH) % S
        else:
            xb = x[b]
            rot = lambda t: t
        offs_c = rot(kidx[q0 : q0 + SH])  # [2048, 32]
        blocks = []
        for qt in range(NQT):
            for cc in range(8):
                O = offs_c[qt * P : (qt + 1) * P, cc * 4 : (cc + 1) * 4]
                I = np.ascontiguousarray(O.T).reshape(-1)  # I[j*128+p]
                blocks.append(I[ppidx])
        offs_dev = np.ascontiguousarray(
            np.concatenate(blocks, axis=1)
        ).astype(np.int16)
        m = dict(shared)
        m["x"] = np.ascontiguousarray(xb)
        m["xbf"] = np.ascontiguousarray(xb.astype(ml_dtypes.bfloat16))
        m["offs"] = offs_dev
        in_maps.append(m)
    return in_maps


def _install_trace_hook():
    import types
    import antenv

    if hasattr(antenv, "axon_hooks"):
        return
    mod = types.ModuleType("antenv.axon_hooks")
    mod._hook = None
    mod.set_axon_ntff_profile_hook = lambda h: setattr(mod, "_hook", h)
    mod.get_axon_ntff_profile_hook = lambda: mod._hook
    sys.modules["antenv.axon_hooks"] = mod
    antenv.axon_hooks = mod
    if "/root/.axon_site" not in sys.path:
        sys.path.insert(0, "/root/.axon_site")
    try:
        from trn_agent_boot.trn_boot import _ntff_profile_via_ctypes

        hook = _ntff_profile_via_ctypes("/opt/axon/libaxon_pjrt.so")
        if hook is not None:
            mod.set_axon_ntff_profile_hook(hook)
    except Exception:
        pass


def kernel(**inputs):
    global _compiled, LAST_RESULT
    from concourse.bass_utils import run_bass_kernel_spmd

    zb = all(
        not np.any(np.asarray(inputs[k], np.float32))
        for k in ("bq", "bk", "bv", "bo", "b2")
    )
    zg = (
        bool(np.all(np.asarray(inputs["ln1_g"], np.float32) == 1.0))
        and bool(np.all(np.asarray(inputs["ln2_g"], np.float32) == 1.0))
        and not np.any(np.asarray(inputs["ln1_b"], np.float32))
        and not np.any(np.asarray(inputs["ln2_b"], np.float32))
    )
    if _compiled is None or _compiled[1] != (zb, zg):
        _compiled = (_build(zb=zb, zg=zg), (zb, zg))
    in_maps = _prep(inputs)
    trace = bool(int(os.environ.get("BASS_KERNEL_TRACE", "0")))
    if trace:
        _install_trace_hook()
    res = run_bass_kernel_spmd(_compiled[0], in_maps, list(range(N_CORES)), trace=trace)
    LAST_RESULT = res
    out = np.empty((B, S, D), np.float32)
    for c in range(N_CORES):
        b, sh = c // 2, c % 2
        out[b, sh * SH : (sh + 1) * SH] = res.results[c]["out"]
    return out


# revision 24
# speedup vs baseline: 1.1534x; 1.1534x over previous
"""Trainium2 Bass kernel for nn_EncoderLayer_85100482003492 (sparse graph attention).

Sharding: 8 cores = (batch b in 0..3) x (query-half sh in 0..1).
Each core handles batch b, queries [sh*2048, (sh+1)*2048), ALL 8 heads:
  - computes K,V for all 4096 tokens of its batch (dup across the pair),
    stores them interleaved as bf16 rows kv[t] = [K(512)|V(512)] in DRAM,
  - indirect-DMA gathers the 32 neighbor KV rows per query (8 x 512-row
    calls per 128-query tile; with 32KB SWDGE descriptor scratch two calls
    fit the ring so desc-gen overlaps the previous call's drain),
  - fused per-query-tile pipeline: dot-products + score tree on DVE (bf16
    2x, in-place in the gather buffer), per-half exp-broadcast expansion on
    ACT (x0.125 scale, starts as soon as that half's scores are ready),
    weighted V sum + tree on DVE (in-place), ctx normalized once by 1/den,
    then WO matmul + LN1 + FFN + LN2 in the same loop so PE/ACT work
    overlaps the next tile's gather DMA.
Host-detected fast paths: when all biases are zero (zb) and LN gamma/beta
are identity (zg) - always true for this problem's setup_inputs - bias
adds become ACT copies and the LN affine folds into the ACT normalize op.
The general path is kept and selected automatically otherwise.
No collectives: each core's output rows are disjoint; host concatenates.
"""
import os
import sys

sys.path.insert(0, "/opt/trn_rl_repo")

import numpy as np

B, S, D, H, DFF, DEG = 4, 4096, 512, 8, 2048, 32
DH = D // H
SH = S // 2          # queries per core
P = 128
NQT = SH // P        # 16 query tiles per core
NTT = S // P         # 32 token tiles
HJ = DEG // 2        # 16 neighbors per gather half
EPS = 1e-6
N_CORES = 8

_compiled = None
LAST_RESULT = None


def _build(zb=False, zg=False):
    import concourse.bacc as bacc
    import concourse.mybir as mybir
    import concourse.tile as tile
    from concourse.library_config import mlp
    from concourse.masks import make_identity

    f32 = mybir.dt.float32
    bf16 = mybir.dt.bfloat16
    ALU = mybir.AluOpType
    ACTF = mybir.ActivationFunctionType

    nc = bacc.Bacc("TRN2", target_bir_lowering=False, debug=False,
                   dynamic_dma_scratch_size=32768)

    x = nc.dram_tensor("x", [S, D], f32, kind="ExternalInput")
    offs = nc.dram_tensor("offs", [P, NQT * 2 * P], mybir.dt.int16, kind="ExternalInput")
    xbf = nc.dram_tensor("xbf", [S, D], bf16, kind="ExternalInput")
    wq = nc.dram_tensor("wq", [D, D], bf16, kind="ExternalInput")
    wk = nc.dram_tensor("wk", [D, D], bf16, kind="ExternalInput")
    wv = nc.dram_tensor("wv", [D, D], bf16, kind="ExternalInput")
    wo = nc.dram_tensor("wo", [D, D], bf16, kind="ExternalInput")
    w1 = nc.dram_tensor("w1", [D, DFF], bf16, kind="ExternalInput")
    w2 = nc.dram_tensor("w2", [DFF, D], bf16, kind="ExternalInput")
    # host-prebroadcast bias/ln tensors
    bq_b = nc.dram_tensor("bq_b", [P, D], f32, kind="ExternalInput")
    bk_b = nc.dram_tensor("bk_b", [P, D], f32, kind="ExternalInput")
    bv_b = nc.dram_tensor("bv_b", [P, D], f32, kind="ExternalInput")
    bo_b = nc.dram_tensor("bo_b", [P, D], f32, kind="ExternalInput")
    b2_b = nc.dram_tensor("b2_b", [P, D], f32, kind="ExternalInput")
    g1_b = nc.dram_tensor("g1_b", [P, D], f32, kind="ExternalInput")
    bt1_b = nc.dram_tensor("bt1_b", [P, D], f32, kind="ExternalInput")
    g2_b = nc.dram_tensor("g2_b", [P, D], f32, kind="ExternalInput")
    bt2_b = nc.dram_tensor("bt2_b", [P, D], f32, kind="ExternalInput")
    b1t = nc.dram_tensor("b1t", [P, DFF // P], f32, kind="ExternalInput")

    out = nc.dram_tensor("out", [SH, D], f32, kind="ExternalOutput")

    nc.gpsimd.load_library(mlp)
    with tile.TileContext(nc) as tc:
        with (
            tc.tile_pool(name="dram", bufs=1, space="DRAM") as dram_pool,
            tc.tile_pool(name="persist", bufs=1) as persist,
        ):
            kv_dram = dram_pool.tile([S, 2 * D], bf16)
            q_dram = dram_pool.tile([SH, D], bf16)

            ident = persist.tile([P, P], bf16)
            make_identity(nc, ident[:])
            eps_t = persist.tile([P, 1], f32)
            nc.vector.memset(eps_t[:], EPS)
            # persistent weights for the fused attention+FFN loop
            wo_s = persist.tile([P, 4, D], bf16)
            nc.sync.dma_start(
                out=wo_s[:], in_=wo.ap()[:].rearrange("(a p) d -> p a d", p=P)
            )
            w1_s = persist.tile([P, 4, DFF], bf16)
            nc.sync.dma_start(
                out=w1_s[:], in_=w1.ap()[:].rearrange("(a p) f -> p a f", p=P)
            )
            w2_s = persist.tile([P, 16, D], bf16)
            nc.sync.dma_start(
                out=w2_s[:], in_=w2.ap()[:].rearrange("(a p) d -> p a d", p=P)
            )
            bos = b2s = g1s = bt1s = g2s = bt2s = None
            if not zb:
                bos = persist.tile([P, D], f32)
                b2s = persist.tile([P, D], f32)
                nc.sync.dma_start(out=bos[:], in_=bo_b.ap()[:])
                nc.sync.dma_start(out=b2s[:], in_=b2_b.ap()[:])
            if not zg:
                g1s = persist.tile([P, D], f32)
                bt1s = persist.tile([P, D], f32)
                g2s = persist.tile([P, D], f32)
                bt2s = persist.tile([P, D], f32)
                nc.sync.dma_start(out=g1s[:], in_=g1_b.ap()[:])
                nc.sync.dma_start(out=bt1s[:], in_=bt1_b.ap()[:])
                nc.sync.dma_start(out=g2s[:], in_=g2_b.ap()[:])
                nc.sync.dma_start(out=bt2s[:], in_=bt2_b.ap()[:])
            b1t_s = persist.tile([P, DFF // P], f32)
            nc.sync.dma_start(out=b1t_s[:], in_=b1t.ap()[:])

            # ---------------- Phase 1: xT, QKV projections, KV store -------
            with (
                tc.tile_pool(name="p1sb", bufs=3) as p1sb,
                tc.tile_pool(name="p1w", bufs=1) as p1w,
                tc.tile_pool(name="p1psmm", bufs=2, space="PSUM") as p1psmm,
            ):
                wq_s = p1w.tile([P, 4, D], bf16)
                wk_s = p1w.tile([P, 4, D], bf16)
                wv_s = p1w.tile([P, 4, D], bf16)
                nc.sync.dma_start(
                    out=wq_s[:], in_=wq.ap()[:].rearrange("(a p) d -> p a d", p=P)
                )
                nc.sync.dma_start(
                    out=wk_s[:], in_=wk.ap()[:].rearrange("(a p) d -> p a d", p=P)
                )
                nc.sync.dma_start(
                    out=wv_s[:], in_=wv.ap()[:].rearrange("(a p) d -> p a d", p=P)
                )
                bqs = p1w.tile([P, D], f32)
                bks = p1w.tile([P, D], f32)
                bvs = p1w.tile([P, D], f32)
                nc.sync.dma_start(out=bqs[:], in_=bq_b.ap()[:])
                nc.sync.dma_start(out=bks[:], in_=bk_b.ap()[:])
                nc.sync.dma_start(out=bvs[:], in_=bv_b.ap()[:])

                xT = p1w.tile([P, 4, S], bf16)  # [d%128, d//128, t]
                for dt in range(4):
                    nc.sync.dma_start(
                        out=xT[:, dt, :],
                        in_=xbf.ap()[:, dt * P : (dt + 1) * P],
                        transpose=True,
                    )

                for tt in range(NTT):
                    kv_stage = p1sb.tile([P, 2 * D], bf16, tag="kvst")
                    kps = p1psmm.tile([P, D], f32, tag="kps")
                    for dt in range(4):
                        nc.tensor.matmul(
                            out=kps[:],
                            lhsT=xT[:, dt, tt * P : (tt + 1) * P],
                            rhs=wk_s[:, dt, :],
                            start=(dt == 0),
                            stop=(dt == 3),
                        )
                    if zb:
                        nc.scalar.copy(out=kv_stage[:, 0:D], in_=kps[:])
                    else:
                        nc.vector.tensor_tensor(
                            out=kv_stage[:, 0:D], in0=kps[:], in1=bks[:], op=ALU.add
                        )
                    vps = p1psmm.tile([P, D], f32, tag="kps")
                    for dt in range(4):
                        nc.tensor.matmul(
                            out=vps[:],
                            lhsT=xT[:, dt, tt * P : (tt + 1) * P],
                            rhs=wv_s[:, dt, :],
                            start=(dt == 0),
                            stop=(dt == 3),
                        )
                    if zb:
                        nc.scalar.copy(out=kv_stage[:, D : 2 * D], in_=vps[:])
                    else:
                        nc.vector.tensor_tensor(
                            out=kv_stage[:, D : 2 * D], in0=vps[:], in1=bvs[:],
                            op=ALU.add,
                        )
                    nc.sync.dma_start(
                        out=kv_dram[tt * P : (tt + 1) * P, :], in_=kv_stage[:]
                    )

                # Q for own half (token tiles [16, 32) after host rotation)
                for qt in range(NQT):
                    tcol = _Q0_TILE + qt
                    qps = p1psmm.tile([P, D], f32, tag="kps")
                    for dt in range(4):
                        nc.tensor.matmul(
                            out=qps[:],
                            lhsT=xT[:, dt, tcol * P : (tcol + 1) * P],
                            rhs=wq_s[:, dt, :],
                            start=(dt == 0),
                            stop=(dt == 3),
                        )
                    q_stage = p1sb.tile([P, D], bf16, tag="qst")
                    if zb:
                        nc.scalar.copy(out=q_stage[:], in_=qps[:])
                    else:
                        nc.vector.tensor_tensor(
                            out=q_stage[:], in0=qps[:], in1=bqs[:], op=ALU.add
                        )
                    nc.sync.dma_start(
                        out=q_dram[qt * P : (qt + 1) * P, :], in_=q_stage[:]
                    )

            # ---------------- Fused attention + FFN loop --------------------
            with (
                tc.tile_pool(name="akv", bufs=3) as akv,
                tc.tile_pool(name="ap64", bufs=2) as ap64,
                tc.tile_pool(name="asmall", bufs=2) as asmall,
                tc.tile_pool(name="abig", bufs=1) as abig,
                tc.tile_pool(name="aps", bufs=2, space="PSUM") as aps,
                tc.tile_pool(name="apsmm", bufs=1, space="PSUM") as apsmm,
                tc.tile_pool(name="bpsh", bufs=3, space="PSUM") as bpsh,
                tc.tile_pool(name="bpsmm", bufs=2, space="PSUM") as bpsmm,
            ):
                for qt in range(NQT):
                    # -------- async KV gathers for this tile (2KB rows) -----
                    offs_t = asmall.tile([P, 4 * 64], mybir.dt.int16, tag="offs")
                    nc.sync.dma_start(
                        out=offs_t[:],
                        in_=offs.ap()[:, qt * 256 : (qt + 1) * 256],
                    )
                    kvgs = []
                    for hf in range(2):
                        kvg = akv.tile([P, HJ, 2 * D], bf16, tag="kvg")
                        for s in range(4):
                            cc = hf * 4 + s
                            nc.gpsimd.dma_gather(
                                kvg[:, s * 4 : (s + 1) * 4, :],
                                kv_dram[:],
                                offs_t[:, cc * 32 : (cc + 1) * 32],
                                P * 4,
                                P * 4,
                                2 * D,
                            )
                        kvgs.append(kvg)

                    q_t = asmall.tile([P, D], bf16, tag="qt")
                    nc.sync.dma_start(
                        out=q_t[:], in_=q_dram[qt * P : (qt + 1) * P, :]
                    )
                    x_t = asmall.tile([P, D], f32, tag="xres")
                    nc.sync.dma_start(
                        out=x_t[:],
                        in_=x.ap()[_Q0_TILE * P + qt * P : _Q0_TILE * P + (qt + 1) * P, :],
                    )
                    if not zb:
                        xpbo = abig.tile([P, D], f32, tag="xpbo")
                        nc.vector.tensor_tensor(
                            out=xpbo[:], in0=x_t[:], in1=bos[:], op=ALU.add
                        )

                    scores = asmall.tile([P, 2, P], f32, tag="scores")
                    for hf in range(2):
                        kvg = kvgs[hf]
                        # prod = Kg * q, in-place into the gathered K half
                        nc.vector.tensor_tensor(
                            out=kvg[:, :, 0:D],
                            in0=kvg[:, :, 0:D],
                            in1=q_t[:]
                            .rearrange("p (o d) -> p o d", o=1)
                            .to_broadcast([P, HJ, D]),
                            op=ALU.mult,
                        )
                        # in-place tree-reduce over dh=64 -> [P, j, h]
                        cur = kvg[:, :, 0:D].rearrange("p j (g d) -> p j g d", d=DH)
                        w = DH
                        while w > 2:
                            half = w // 2
                            nc.vector.tensor_tensor(
                                out=cur[:, :, :, 0:half],
                                in0=cur[:, :, :, 0:half],
                                in1=cur[:, :, :, half:w],
                                op=ALU.add,
                            )
                            w = half
                        nc.vector.tensor_tensor(
                            out=scores[:, hf, :].rearrange(
                                "p (j g o) -> p j g o", g=H, o=1
                            ),
                            in0=cur[:, :, :, 0:1],
                            in1=cur[:, :, :, 1:2],
                            op=ALU.add,
                        )

                    # per-half: e64 = exp-broadcast (starts as soon as this
                    # half's scores are done), weighted V (in-place), tree
                    den = asmall.tile([P, H], f32, tag="den")
                    den_h = asmall.tile([P, H], f32, tag="denh")
                    ctx_halves = []
                    for hf in range(2):
                        kvg = kvgs[hf]
                        e64 = ap64.tile([P, HJ * H, DH], bf16, tag="p64")
                        nc.scalar.activation(
                            out=e64[:],
                            in_=scores[:, hf, :]
                            .rearrange("p (a o) -> p a o", o=1)
                            .to_broadcast([P, HJ * H, DH]),
                            func=ACTF.Exp,
                            scale=0.125,
                        )
                        nc.vector.tensor_reduce(
                            out=(den if hf == 0 else den_h)[:],
                            in_=e64[:]
                            .rearrange("p (j g) d -> p j g d", g=H)[:, :, :, 0:1]
                            .rearrange("p j g o -> p g (j o)"),
                            axis=mybir.AxisListType.X,
                            op=ALU.add,
                        )
                        nc.vector.tensor_tensor(
                            out=kvg[:, :, D : 2 * D],
                            in0=kvg[:, :, D : 2 * D],
                            in1=e64[:].rearrange("p (j g) d -> p j (g d)", g=H),
                            op=ALU.mult,
                        )
                        w = HJ
                        while w > 2:
                            half = w // 2
                            nc.vector.tensor_tensor(
                                out=kvg[:, 0:half, D : 2 * D],
                                in0=kvg[:, 0:half, D : 2 * D],
                                in1=kvg[:, half:w, D : 2 * D],
                                op=ALU.add,
                            )
                            w = half
                        ctx_halves.append(kvg)
                    nc.vector.tensor_tensor(
                        out=den[:], in0=den[:], in1=den_h[:], op=ALU.add
                    )
                    rden = asmall.tile([P, H], f32, tag="rden")
                    nc.vector.reciprocal(out=rden[:], in_=den[:])

                    ctx_n = abig.tile([P, D], bf16, tag="ctxn")
                    nc.vector.tensor_tensor(
                        out=ctx_n[:],
                        in0=ctx_halves[0][:, 0, D : 2 * D],
                        in1=ctx_halves[0][:, 1, D : 2 * D],
                        op=ALU.add,
                    )
                    nc.vector.tensor_tensor(
                        out=ctx_n[:],
                        in0=ctx_n[:],
                        in1=ctx_halves[1][:, 0, D : 2 * D],
                        op=ALU.add,
                    )
                    nc.vector.tensor_tensor(
                        out=ctx_n[:],
                        in0=ctx_n[:],
                        in1=ctx_halves[1][:, 1, D : 2 * D],
                        op=ALU.add,
                    )
                    nc.vector.tensor_tensor(
                        out=ctx_n[:].rearrange("p (g d) -> p g d", d=DH),
                        in0=ctx_n[:].rearrange("p (g d) -> p g d", d=DH),
                        in1=rden[:]
                        .rearrange("p (g o) -> p g o", o=1)
                        .to_broadcast([P, H, DH]),
                        op=ALU.mult,
                    )

                    # transpose ctx, WO matmul, residual, LN1
                    ctxT = abig.tile([P, 4, P], bf16, tag="ctxT")
                    for dt in range(4):
                        tp = aps.tile([P, P], bf16, tag="tp")
                        nc.tensor.transpose(
                            out=tp[:],
                            in_=ctx_n[:, dt * P : (dt + 1) * P],
                            identity=ident[:],
                        )
                        nc.scalar.copy(out=ctxT[:, dt, :], in_=tp[:])
                    attn = apsmm.tile([P, D], f32, tag="attn")
                    for dt in range(4):
                        nc.tensor.matmul(
                            out=attn[:],
                            lhsT=ctxT[:, dt, :],
                            rhs=wo_s[:, dt, :],
                            start=(dt == 0),
                            stop=(dt == 3),
                        )
                    x1pre = abig.tile([P, D], f32, tag="x1pre")
                    nc.vector.tensor_tensor(
                        out=x1pre[:], in0=attn[:],
                        in1=x_t[:] if zb else xpbo[:], op=ALU.add
                    )
                    x1 = abig.tile([P, D], f32, tag="x1")
                    _layernorm(nc, tc, abig, x1[:], x1pre[:], None if zg else g1s[:], None if zg else bt1s[:], eps_t, ALU, ACTF, f32, identity_gb=zg)

                    # FFN (transposed h layout) + residual + LN2
                    x1b = abig.tile([P, D], bf16, tag="x1b")
                    nc.scalar.copy(out=x1b[:], in_=x1[:])
                    x1T = abig.tile([P, 4, P], bf16, tag="x1T")
                    for dt in range(4):
                        tp = aps.tile([P, P], bf16, tag="tp")
                        nc.tensor.transpose(
                            out=tp[:],
                            in_=x1b[:, dt * P : (dt + 1) * P],
                            identity=ident[:],
                        )
                        nc.scalar.copy(out=x1T[:, dt, :], in_=tp[:])
                    hT = abig.tile([P, 16, P], bf16, tag="hT")
                    for ft in range(16):
                        hps = bpsh.tile([P, P], f32, tag="hps")
                        for dt in range(4):
                            nc.tensor.matmul(
                                out=hps[:],
                                lhsT=w1_s[:, dt, ft * P : (ft + 1) * P],
                                rhs=x1T[:, dt, :],
                                start=(dt == 0),
                                stop=(dt == 3),
                            )
                        nc.scalar.activation(
                            out=hT[:, ft, :],
                            in_=hps[:],
                            func=ACTF.Relu,
                            bias=b1t_s[:, ft : ft + 1],
                        )
                    y2 = bpsmm.tile([P, D], f32, tag="y2")
                    for ft in range(16):
                        nc.tensor.matmul(
                            out=y2[:],
                            lhsT=hT[:, ft, :],
                            rhs=w2_s[:, ft, :],
                            start=(ft == 0),
                            stop=(ft == 15),
                        )
                    x2pre = abig.tile([P, D], f32, tag="x2pre")
                    if zb:
                        nc.vector.tensor_tensor(
                            out=x2pre[:], in0=y2[:], in1=x1[:], op=ALU.add
                        )
                    else:
                        nc.vector.tensor_tensor(
                            out=x2pre[:], in0=y2[:], in1=b2s[:], op=ALU.add
                        )
                        nc.vector.tensor_tensor(
                            out=x2pre[:], in0=x2pre[:], in1=x1[:], op=ALU.add
                        )
                    o_t = abig.tile([P, D], f32, tag="ot")
                    _layernorm(nc, tc, abig, o_t[:], x2pre[:], None if zg else g2s[:], None if zg else bt2s[:], eps_t, ALU, ACTF, f32, identity_gb=zg)
                    nc.sync.dma_start(
                        out=out.ap()[qt * P : (qt + 1) * P, :], in_=o_t[:]
                    )

    nc.compile()
    return nc


def _layernorm(nc, tc, pool, out_ap, in_ap, g_b, bt_b, eps_t, ALU, ACTF, f32,
               identity_gb=False):
    """out = (in - mean)/sqrt(var+EPS) * g + b, per partition over D columns."""
    import concourse.mybir as mybir

    s1 = pool.tile([P, 1], f32, tag="ln_s1")
    nc.vector.tensor_reduce(
        out=s1[:], in_=in_ap, axis=mybir.AxisListType.X, op=ALU.add
    )
    sqd = pool.tile([P, D], f32, tag="ln_scratch")
    s2 = pool.tile([P, 1], f32, tag="ln_s2")
    nc.scalar.activation(
        out=sqd[:], in_=in_ap, func=ACTF.Square, accum_out=s2[:]
    )
    nmean = pool.tile([P, 1], f32, tag="ln_nmean")
    nc.scalar.mul(out=nmean[:], in_=s1[:], mul=-1.0 / D)
    ex2 = pool.tile([P, 1], f32, tag="ln_ex2")
    nc.scalar.mul(out=ex2[:], in_=s2[:], mul=1.0 / D)
    m2 = pool.tile([P, 1], f32, tag="ln_m2")
    nc.vector.tensor_tensor(out=m2[:], in0=nmean[:], in1=nmean[:], op=ALU.mult)
    var = pool.tile([P, 1], f32, tag="ln_var")
    nc.vector.tensor_tensor(out=var[:], in0=ex2[:], in1=m2[:], op=ALU.subtract)
    std = pool.tile([P, 1], f32, tag="ln_std")
    nc.scalar.activation(out=std[:], in_=var[:], func=ACTF.Sqrt, bias=eps_t[:, 0:1])
    rstd = pool.tile([P, 1], f32, tag="ln_rstd")
    nc.vector.reciprocal(out=rstd[:], in_=std[:])
    nmr = pool.tile([P, 1], f32, tag="ln_nmr")
    nc.vector.tensor_tensor(out=nmr[:], in0=nmean[:], in1=rstd[:], op=ALU.mult)
    if identity_gb:
        nc.scalar.activation(
            out=out_ap, in_=in_ap, func=ACTF.Identity, bias=nmr[:, 0:1],
            scale=rstd[:, 0:1],
        )
    else:
        xn = pool.tile([P, D], f32, tag="ln_scratch")
        nc.scalar.activation(
            out=xn[:], in_=in_ap, func=ACTF.Identity, bias=nmr[:, 0:1],
            scale=rstd[:, 0:1],
        )
        nc.vector.tensor_tensor(out=xn[:], in0=xn[:], in1=g_b, op=ALU.mult)
        nc.vector.tensor_tensor(out=out_ap, in0=xn[:], in1=bt_b, op=ALU.add)


# Q-tile offset within the 32 token tiles. Both half-cores share the same
# compiled program; the host passes x ROTATED for sh=0 cores so that the
# query half always sits at token tiles [16, 32). See _prep().
_Q0_TILE = 16


def _prep(inputs):
    x = np.ascontiguousarray(np.asarray(inputs["x"], dtype=np.float32))
    edges = np.asarray(inputs["edges"])
    kidx = np.ascontiguousarray(edges[:, 1].reshape(S, DEG)).astype(np.int32)

    def bb(name):
        return np.ascontiguousarray(
            np.broadcast_to(np.asarray(inputs[name], np.float32), (P, D))
        )

    import ml_dtypes

    def cbf(name):
        return np.ascontiguousarray(
            np.asarray(inputs[name], np.float32).astype(ml_dtypes.bfloat16)
        )

    shared = {
        "wq": cbf("wq"),
        "wk": cbf("wk"),
        "wv": cbf("wv"),
        "wo": cbf("wo"),
        "w1": cbf("w1"),
        "w2": cbf("w2"),
        "bq_b": bb("bq"),
        "bk_b": bb("bk"),
        "bv_b": bb("bv"),
        "bo_b": bb("bo"),
        "b2_b": bb("b2"),
        "g1_b": bb("ln1_g"),
        "bt1_b": bb("ln1_b"),
        "g2_b": bb("ln2_g"),
        "bt2_b": bb("ln2_b"),
        "b1t": np.ascontiguousarray(
            np.asarray(inputs["b1"], np.float32).reshape(DFF // P, P).T
        ),
    }

    # dma_gather wrapped idx layout for a 512-row gather block (qt, hf, s):
    # gathered row i = edge (q = i%128, j = hf*16 + s*4 + i//128); idx value
    # for row i sits at [partition i%16, column i//16], replicated x8 cores.
    # 512-row calls (1024 descriptors) let two calls coexist in the 2048-slot
    # SWDGE ring so desc-gen overlaps the previous call's drain.
    ppidx = (np.arange(32)[None, :] * 16) + (np.arange(P)[:, None] % 16)

    in_maps = []
    for c in range(N_CORES):
        b, sh = c // 2, c % 2
        q0 = sh * SH
        # rotate tokens so this core's queries sit at token tiles [16, 32)
        # (kv gather indices are rotated to match)
        if sh == 0:
            xb = np.concatenate([x[b, SH:], x[b, :SH]], axis=0)
            rot = lambda t: (t + SH) % S
        else:
            xb = x[b]
            rot = lambda t: t
        offs_c = rot(kidx[q0 : q0 + SH])  # [2048, 32]
        blocks = []
        for qt in range(NQT):
            for cc in range(8):
                O = offs_c[qt * P : (qt + 1) * P, cc * 4 : (cc + 1) * 4]
                I = np.ascontiguousarray(O.T).reshape(-1)  # I[j*128+p]
                blocks.append(I[ppidx])
        offs_dev = np.ascontiguousarray(
            np.concatenate(blocks, axis=1)
        ).astype(np.int16)
        m = dict(shared)
        m["x"] = np.ascontiguousarray(xb)
        m["xbf"] = np.ascontiguousarray(xb.astype(ml_dtypes.bfloat16))
        m["offs"] = offs_dev
        in_maps.append(m)
    return in_maps


def _install_trace_hook():
    import types
    import antenv

    if hasattr(antenv, "axon_hooks"):
        return
    mod = types.ModuleType("antenv.axon_hooks")
    mod._hook = None
    mod.set_axon_ntff_profile_hook = lambda h: setattr(mod, "_hook", h)
    mod.get_axon_ntff_profile_hook = lambda: mod._hook
    sys.modules["antenv.axon_hooks"] = mod
    antenv.axon_hooks = mod
    if "/root/.axon_site" not in sys.path:
        sys.path.insert(0, "/root/.axon_site")
    try:
        from trn_agent_boot.trn_boot import _ntff_profile_via_ctypes

        hook = _ntff_profile_via_ctypes("/opt/axon/libaxon_pjrt.so")
        if hook is not None:
            mod.set_axon_ntff_profile_hook(hook)
    except Exception:
        pass


def kernel(**inputs):
    global _compiled, LAST_RESULT
    from concourse.bass_utils import run_bass_kernel_spmd

    zb = all(
        not np.any(np.asarray(inputs[k], np.float32))
        for k in ("bq", "bk", "bv", "bo", "b2")
    )
    zg = (
        bool(np.all(np.asarray(inputs["ln1_g"], np.float32) == 1.0))
        and bool(np.all(np.asarray(inputs["ln2_g"], np.float32) == 1.0))
        and not np.any(np.asarray(inputs["ln1_b"], np.float32))
        and not np.any(np.asarray(inputs["ln2_b"], np.float32))
    )
    if _compiled is None or _compiled[1] != (zb, zg):
        _compiled = (_build(zb=zb, zg=zg), (zb, zg))
    in_maps = _prep(inputs)
    trace = bool(int(os.environ.get("BASS_KERNEL_TRACE", "0")))
    if trace:
        _install_trace_hook()
    res = run_bass_kernel_spmd(_compiled[0], in_maps, list(range(N_CORES)), trace=trace)
    LAST_RESULT = res
    out = np.empty((B, S, D), np.float32)
    for c in range(N_CORES):
        b, sh = c // 2, c % 2
        out[b, sh * SH : (sh + 1) * SH] = res.results[c]["out"]
    return out
